# revision 24
# baseline (speedup 1.0000x reference)
"""Bass/Trainium2 kernel for windowed multi-head attention (8 NeuronCores).

Math per window b (64 tokens), matching torch-style nn.MultiHeadAttention:
  qh = (q @ Wq.T + bq) * scale; kh = k @ Wk.T + bk; vh = v @ Wv.T + bv
  S  = qh @ kh.T  (per head);  S[masked k] = -1e4;  P = softmax(S)
  out = concat_h(P @ vh) @ Wp.T + bp

Device dataflow (per core, 256 windows = 16384 tokens).  This revision
computes attention TRANSPOSED (S.T with key-tokens on partitions) which
removes the mask matmul, all PE transposes, the P.T ACT copy and the DVE
row-sum reduce of the previous design:
  - q,k,v pre-transposed on host to feature-major [128, 4, TT] blocks.
  - q,k and Wq,Wk are fp8(e4m3) DoubleRow (host prescale by 64; q also
    by softmax SCALE; PSUM->SBUF copies descale by 1/64).
  - k -> plain feature-major ykT; q -> zero-padded per-head-slot yqTz
    (the padding keeps the two heads of a chunk separate in the K=128
    S.T matmuls); v -> token-major yv.
  - Per window-pair: S.T[tk, (h, tq)] via 8 K=128 matmuls; mask applied
    as the per-partition bias AP of the Exp activation (tk is the
    partition axis, so the additive mask is a [128,1] vector); softmax
    denominators computed TRANSPOSED (denT[(par,tq),(ch,w)] via 4 F=2
    matmuls with the pexpT slab stationary) so the reciprocal is a
    cheap [128,8] DVE op (reciprocal runs ~8 cycles/elem/lane - a
    [2,512] layout would cost 3.3us); the reciprocals are PE-transposed
    to [8,128] and re-broadcast across window halves with 4 K=8
    selector matmuls; P.T is normalized by one DVE multiply, its odd
    half relocated to partition base 0 by a gpsimd copy, then X.T via
    16 64x64 matmuls and the output projection from X.T chunks; output
    stored bf16 (host casts to f32).
  - The whole thing is software-pipelined: a 3-stage pair pipeline
    (front = S/exp/denominators, mid = broadcast/normalize/X.T,
    back = output projection/store, emitted back-front-mid) with the
    NEXT tile's projection matmuls interleaved between pair slots so
    the PE queue never drains (an empty PE window re-throttles the HAM
    clock gate to 1.2 GHz for ~3.4us, which is why engine-idle gaps
    cost roughly double their length).
  - PSUM->SBUF drains are balanced across ACT (k copies, q even-half
    copies, exp, X.T drain) and DVE (v copies, q odd-half copies,
    reciprocal, normalize, out drain): the two drain engines together
    are the co-bottleneck with the PE (each 512-col drain costs
    ~(N+352)/1.2 ns on ACT / ~(N+130)/0.96 on DVE regardless of the
    partition count).

All matmul operands sit at partition base 0 with K in {64, 128, 256}:
mixing row-group offsets across back-to-back K<128 matmuls crashes the
PE, and DoubleRow matmuls additionally require dst partition base 0.

Biases are zero in this problem; nonzero bv/bp fold into a host-side
output add, nonzero bq/bk fall back to a host computation.
"""

import sys

for _p in ("/opt/trn_rl_repo",):
    if _p not in sys.path:
        sys.path.append(_p)

import numpy as np
import ml_dtypes

import concourse.bacc as bacc
import concourse.bass as bass
import concourse.mybir as mybir
import concourse.tile as tile
from concourse.bass import ds, ts
from concourse.bass_utils import run_bass_kernel_spmd

BF16 = mybir.dt.bfloat16
FP8 = mybir.dt.float8e4
F32 = mybir.dt.float32
NP_BF16 = ml_dtypes.bfloat16
NP_FP8 = ml_dtypes.float8_e4m3
WSCALE = 64.0  # fp8 weight prescale (host) / copy descale (device)

NCORES = 8
DIM = 512
HEADS = 8
HD = 64
N = 64  # window length
B_TOTAL = 2048
WIN_PER_CORE = B_TOTAL // NCORES  # 256
TOK_PER_CORE = WIN_PER_CORE * N  # 16384
SCALE = HD ** -0.5
MASK_NEG = -10000.0
TT = 512  # token tile (8 windows, 4 pairs)


def build_program(win_per_core=WIN_PER_CORE):
    """Build the per-core Tile program. All 8 cores run it SPMD."""
    tok = win_per_core * N
    n_tt = tok // TT
    PAIRS = TT // 128  # 4

    nc = bacc.Bacc("TRN2", debug=False)

    qT_d = nc.dram_tensor("qT", [n_tt, 128, 4, TT], FP8, kind="ExternalInput")
    kT_d = nc.dram_tensor("kT", [n_tt, 128, 4, TT], FP8, kind="ExternalInput")
    vT_d = nc.dram_tensor("vT", [n_tt, 128, 4, TT], BF16, kind="ExternalInput")
    maskcol = nc.dram_tensor(
        "maskcol", [n_tt, 128, PAIRS], F32, kind="ExternalInput"
    )
    wq = nc.dram_tensor("wq", [DIM, DIM], FP8, kind="ExternalInput")
    wk = nc.dram_tensor("wk", [DIM, DIM], FP8, kind="ExternalInput")
    wv = nc.dram_tensor("wv", [DIM, DIM], BF16, kind="ExternalInput")
    wp = nc.dram_tensor("wp", [DIM, DIM], BF16, kind="ExternalInput")
    ones2 = nc.dram_tensor("ones2", [128, 2], BF16, kind="ExternalInput")
    wsel = nc.dram_tensor("wsel", [4, 8, 128], BF16, kind="ExternalInput")
    ident = nc.dram_tensor("ident", [128, 128], BF16, kind="ExternalInput")
    out = nc.dram_tensor("out", [tok, DIM], BF16, kind="ExternalOutput")

    qa, ka, va, ma = qT_d.ap(), kT_d.ap(), vT_d.ap(), maskcol.ap()
    oa = out.ap()

    with tile.TileContext(nc) as tc:
        with (
            tc.tile_pool(name="consts", bufs=1) as consts,
            tc.tile_pool(name="pin", bufs=3) as pin,
            tc.tile_pool(name="py", bufs=3) as py,
            tc.tile_pool(name="pp", bufs=6) as pp,
            tc.tile_pool(name="pout", bufs=4) as pout,
            tc.tile_pool(name="ps_pj", bufs=3, space="PSUM") as ps_pj,
            tc.tile_pool(name="ps_s", bufs=2, space="PSUM") as ps_s,
            tc.tile_pool(name="ps_x", bufs=2, space="PSUM") as ps_x,
            tc.tile_pool(name="ps_o", bufs=1, space="PSUM") as ps_o,
        ):
            # Weights: [fi_part 128, fi_chunk 4, fo 512]
            w_tiles = {}
            for name, t in (("wq", wq), ("wk", wk), ("wv", wv), ("wp", wp)):
                dt_ = FP8 if name in ("wq", "wk") else BF16
                wt = consts.tile([128, 4, DIM], dt_, tag=f"w_{name}")
                nc.sync.dma_start(
                    out=wt, in_=t.ap().rearrange("(c p) f -> p c f", p=128)
                )
                w_tiles[name] = wt
            # ones2: col w sums window w's partitions; sel2: row w
            # broadcasts to window w's partition half
            ones_t = consts.tile([128, 2], BF16, tag="ones2")
            nc.sync.dma_start(out=ones_t, in_=ones2.ap())
            # wsel[ch][(ch2, w), tkg] = (ch2 == ch) & (tkg // 64 == w):
            # selects this chunk's reciprocal row for each window half
            wsel_t = []
            for ch in range(4):
                wt_ = consts.tile([8, 128], BF16, tag=f"wsel{ch}", name=f"wsel{ch}")
                nc.sync.dma_start(out=wt_, in_=wsel.ap()[ch])
                wsel_t.append(wt_)
            id8_t = consts.tile([128, 128], BF16, tag="id8")
            nc.sync.dma_start(out=id8_t, in_=ident.ap())

            # Two persistent zero-padded q tiles, manually alternated per
            # T-tile: their padding slots are zeroed exactly once and are
            # never overwritten afterwards.
            yqTz_bufs = []
            for i in range(2):
                bz = consts.tile(
                    [128, 4, 2, TT], BF16, tag=f"yqTz{i}", name=f"yqTz{i}"
                )
                nc.gpsimd.memset(bz[0:64, :, 1, :], 0.0)
                nc.gpsimd.memset(bz[64:128, :, 0, :], 0.0)
                yqTz_bufs.append(bz)

            def emit_inputs(t):
                qT = pin.tile([128, 4, TT], FP8, tag="qT", name="qT")
                kT = pin.tile([128, 4, TT], FP8, tag="kT", name="kT")
                vT = pin.tile([128, 4, TT], BF16, tag="vT", name="vT")
                mc = pin.tile([128, PAIRS], F32, tag="mc", name="mc")
                nc.sync.dma_start(out=qT, in_=qa[t])
                nc.sync.dma_start(out=kT, in_=ka[t])
                nc.sync.dma_start(out=vT, in_=va[t])
                nc.sync.dma_start(out=mc, in_=ma[t])
                return qT, kT, vT, mc

            def emit_kproj(kT):
                # k projection -> plain feature-major bf16 [fo 128, c, tok]
                ykT = py.tile([128, 4, TT], BF16, tag="ykT", name="ykT")
                for co in range(4):
                    ps = ps_pj.tile([128, TT], F32, tag="pj", name="ps")
                    for i in range(2):
                        nc.tensor.matmul(
                            ps,
                            lhsT=w_tiles["wk"][:, 2 * i : 2 * i + 2, ts(co, 128)],
                            rhs=kT[:, 2 * i : 2 * i + 2, :],
                            start=(i == 0),
                            stop=(i == 1),
                            perf_mode=mybir.MatmulPerfMode.DoubleRow,
                        )
                    nc.scalar.activation(
                        out=ykT[:, co, :],
                        in_=ps,
                        func=mybir.ActivationFunctionType.Copy,
                        scale=1.0 / WSCALE,
                    )
                return ykT

            def emit_qproj(qT, t):
                # q projection -> zero-padded per-head-slot layout:
                # yqTz[0:64, c, 0, :] = head 2c rows, yqTz[64:128, c, 1, :]
                # = head 2c+1 rows, other slots stay zero so the K=128 S.T
                # matmuls keep the two heads of a chunk separate.  The two
                # PSUM half drains go to different engines so the bank
                # frees at the speed of one drain, not two.
                yqTz = yqTz_bufs[t % 2]
                for co in range(4):
                    ps = ps_pj.tile([128, TT], F32, tag="pj", name="ps")
                    for i in range(2):
                        nc.tensor.matmul(
                            ps,
                            lhsT=w_tiles["wq"][:, 2 * i : 2 * i + 2, ts(co, 128)],
                            rhs=qT[:, 2 * i : 2 * i + 2, :],
                            start=(i == 0),
                            stop=(i == 1),
                            perf_mode=mybir.MatmulPerfMode.DoubleRow,
                        )
                    nc.scalar.activation(
                        out=yqTz[0:64, co, 0, :],
                        in_=ps[0:64, :],
                        func=mybir.ActivationFunctionType.Copy,
                        scale=1.0 / WSCALE,
                    )
                    nc.vector.tensor_scalar_mul(
                        yqTz[64:128, co, 1, :], ps[64:128, :], 1.0 / WSCALE
                    )
                return yqTz

            def emit_vproj_half(vT, yv, cts):
                # v projection -> token-major bf16 [tok%128, pair, fo];
                # emitted in two halves on different pair slots to even
                # out the PE load per slot
                if yv is None:
                    yv = py.tile([128, PAIRS, DIM], BF16, tag="yv", name="yv")
                for ct in cts:
                    ps = ps_pj.tile([128, DIM], F32, tag="pj", name="ps")
                    for ci in range(4):
                        nc.tensor.matmul(
                            ps,
                            lhsT=vT[:, ci, ts(ct, 128)],
                            rhs=w_tiles["wv"][:, ci, :],
                            start=(ci == 0),
                            stop=(ci == 3),
                        )
                    if ct % 2 == 0:
                        nc.scalar.copy(out=yv[:, ct, :], in_=ps)
                    else:
                        nc.vector.tensor_copy(out=yv[:, ct, :], in_=ps)
                return yv

            def emit_vhi(yv):
                # second window's vh rows relocated to partition base 0
                yv_hi = py.tile([64, PAIRS, DIM], BF16, tag="yv_hi", name="yv_hi")
                nc.gpsimd.dma_start(out=yv_hi, in_=yv[64:128, :, :])
                return yv_hi

            # ---- prologue: tile 0 inputs + projections ----
            qT0, kT0, vT0, mc0 = emit_inputs(0)
            yv0 = emit_vproj_half(vT0, None, range(PAIRS))
            proj = {
                0: (
                    emit_kproj(kT0),
                    emit_qproj(qT0, 0),
                    yv0,
                    emit_vhi(yv0),
                    mc0,
                )
            }
            pend_in = None

            # ---- attention pipeline over all (tile, pair) slots ----
            # Three-stage software pipeline (one extra pair of latency each):
            #   front(s):  S.T matmuls, exp, transposed denominators,
            #              reciprocal, PE transpose + [8, 128] drain
            #   mid(s-1):  reciprocal broadcast, normalize P.T, X.T
            #              matmuls, X.T drain
            #   back(s-2): output projection, out drain, store
            # The NEXT tile's projection matmuls + drains are interleaved
            # between pair slots so the PE never drains its queue while the
            # ACT engine catches up (an empty PE window re-throttles the
            # HAM clock gate to 1.2 GHz for ~3.4 us).
            st1 = st2 = None
            total = n_tt * PAIRS + 2
            for s in range(total):
                t, pr = divmod(s, PAIRS)
                if st2 is not None:
                    xts_b, ab2 = st2
                    # output projection: [tok 128, fo 512]
                    po = ps_o.tile([128, DIM], F32, tag="po", name="po")
                    for c in range(4):
                        nc.tensor.matmul(
                            po,
                            lhsT=xts_b[:, c, :],
                            rhs=w_tiles["wp"][:, c, :],
                            start=(c == 0),
                            stop=(c == 3),
                        )
                    osb = pout.tile([128, DIM], BF16, tag="osb", name="osb")
                    nc.vector.tensor_copy(out=osb, in_=po)
                    nc.sync.dma_start(out=oa[ab2 : ab2 + 128, :], in_=osb)
                if st1 is not None:
                    pexpT_b, rec2T_b, yv_b, yv_hi_b, ab = st1
                    prb = (ab % TT) // 128
                    # broadcast reciprocals across the window halves:
                    # rec_bc[tkg, ch, par, tq] = rec2T[(ch, w(tkg)), par, tq]
                    rec_bc = ps_s.tile(
                        [128, 4, 2, N], F32, tag="spT", name="rec_bc"
                    )
                    for ch in range(4):
                        nc.tensor.matmul(
                            rec_bc[:, ch, :, :],
                            lhsT=wsel_t[ch],
                            rhs=rec2T_b,
                            start=True,
                            stop=True,
                            skip_group_check=True,
                        )
                    # normalized P.T (in1 reads the PSUM broadcast directly)
                    pnT = pp.tile([128, HEADS, N], BF16, tag="pnT", name="pnT")
                    nc.vector.tensor_tensor(
                        out=pnT,
                        in0=pexpT_b,
                        in1=rec_bc.rearrange("p c w n -> p (c w) n"),
                        op=mybir.AluOpType.mult,
                    )
                    # odd-window rows relocated to partition base 0 (the
                    # gpsimd DMA path: the HWDGE queues are congested with
                    # input/output traffic while GpSimd is idle)
                    pnT_hi = pp.tile([64, HEADS, N], BF16, tag="pnT_hi", name="pnT_hi")
                    nc.gpsimd.dma_start(out=pnT_hi, in_=pnT[64:128, :, :])
                    # X.T: [feat-in-chunk 128, chunk 4, (w, tq) 128]
                    xt = ps_x.tile([128, 4, 128], F32, tag="xt", name="xt")
                    for h in range(HEADS):
                        ch, hh = h // 2, (h % 2) * 64
                        for w in range(2):
                            psrc = pnT if w == 0 else pnT_hi
                            vsrc = yv_b if w == 0 else yv_hi_b
                            nc.tensor.matmul(
                                xt[ds(hh, 64), ch, ds(64 * w, 64)],
                                lhsT=vsrc[ds(0, 64), prb, ts(h, 64)],
                                rhs=psrc[ds(0, 64), h, :],
                                start=True,
                                stop=True,
                            )
                    xts = pp.tile([128, 4, 128], BF16, tag="xts", name="xts")
                    nc.scalar.copy(out=xts, in_=xt)
                    st2 = (xts, ab)
                else:
                    st2 = None
                nst1 = None
                if t < n_tt:
                    ykT, yqTz, yv, yv_hi, mc = proj[t]
                    base = pr * 128
                    # S.T: [(w, tk) 128, (head, tq) 512].  Each (ch, w)
                    # region is written exactly once -> every matmul is its
                    # own accumulation group (start only clears has_written
                    # for the region it writes).
                    spT = ps_s.tile([128, HEADS, N], F32, tag="spT", name="spT")
                    for ch in range(4):
                        for w in range(2):
                            nc.tensor.matmul(
                                spT[ds(64 * w, 64), ds(2 * ch, 2), :],
                                lhsT=ykT[:, ch, ds(base + 64 * w, 64)],
                                rhs=yqTz[:, ch, :, ds(base + 64 * w, 64)],
                                start=True,
                                stop=True,
                                skip_group_check=True,
                            )
                    # softmax numerator: exp(S.T + mask[tk]); the additive
                    # mask rides the ACT's per-partition bias port.
                    pexpT = pp.tile([128, HEADS, N], BF16, tag="pexpT", name="pexpT")
                    nc.scalar.activation(
                        out=pexpT,
                        in_=spT,
                        func=mybir.ActivationFunctionType.Exp,
                        bias=mc[:, pr : pr + 1],
                    )
                    # transposed denominators denT[(par, tq) 128, (ch, w)]
                    # + the reciprocal transpose share one scratch PSUM bank
                    # (bitcast views), so spT's bank frees right after exp
                    # and the next slot's S.T matmuls are not chained behind
                    # the reciprocal.  This orientation makes the reciprocal
                    # a cheap [128, 8] op (~8 DVE cycles per element/lane).
                    scr = ps_x.tile([128, 1024], BF16, tag="xt", name="scr")
                    denT8 = scr[:, 0:16].bitcast(F32)
                    denT = denT8.rearrange("p (c w) -> p c w", c=4)
                    for ch in range(4):
                        nc.tensor.matmul(
                            denT[:, ch, :],
                            lhsT=pexpT[:, 2 * ch : 2 * ch + 2, :],
                            rhs=ones_t,
                            start=True,
                            stop=True,
                            skip_group_check=True,
                        )
                    rT = pp.tile([128, 8], BF16, tag="rT", name="rT")
                    with nc.allow_low_precision(
                        reason="softmax denominators are O(1..64) sums; "
                        "bf16 reciprocal is inside the fp8 error budget"
                    ):
                        nc.vector.reciprocal(out=rT, in_=denT8)
                    # PE-transpose the [128, 8] reciprocals to [8, 128]
                    # (rows become (ch, w)), then a cheap [8, 128] drain
                    recT_ps = scr[0:8, 128:256]
                    nc.tensor.transpose(recT_ps, rT, id8_t)
                    rec2T = pp.tile([8, 128], BF16, tag="rec2T", name="rec2T")
                    nc.vector.tensor_copy(out=rec2T, in_=recT_ps)
                    nst1 = (pexpT, rec2T, yv, yv_hi, t * TT + base)
                st1 = nst1
                # interleave the next tile's inputs + projections between
                # this tile's pair slots
                if t + 1 < n_tt:
                    if pr == 0:
                        pend_in = emit_inputs(t + 1)
                        pend_k = emit_kproj(pend_in[1])
                    elif pr == 1:
                        pend_q = emit_qproj(pend_in[0], t + 1)
                    elif pr == 2:
                        pend_v = emit_vproj_half(pend_in[2], None, range(PAIRS))
                    elif pr == 3:
                        proj[t + 1] = (
                            pend_k,
                            pend_q,
                            pend_v,
                            emit_vhi(pend_v),
                            pend_in[3],
                        )
                        del proj[t]

    nc.compile()
    return nc


_PROGRAM_CACHE = {}


def _get_program(win_per_core):
    if win_per_core not in _PROGRAM_CACHE:
        _PROGRAM_CACHE[win_per_core] = build_program(win_per_core)
    return _PROGRAM_CACHE[win_per_core]


def _feature_major_tiles(x_flat_bf16):
    """[tok, 512] -> [n_tt, 128, 4, TT] so each T-tile block is one
    fully-contiguous transposed DMA read."""
    tok = x_flat_bf16.shape[0]
    n_tt = tok // TT
    xt = x_flat_bf16.reshape(n_tt, TT, 4, 128).transpose(0, 3, 2, 1)
    return np.ascontiguousarray(xt)


def make_in_maps(q, k, v, mask, Wq, Wk, Wv, Wp, ncores=NCORES):
    """Host-side shard + layout prep. Returns list of per-core input dicts."""
    B, n, C = q.shape
    win_pc = B // ncores
    n_tt = win_pc * n // TT
    qf = np.ascontiguousarray(q.reshape(B * n, C))
    kf = np.ascontiguousarray(k.reshape(B * n, C))
    vf = np.ascontiguousarray(v.reshape(B * n, C))

    # additive mask as a per-key-token column: [B(win), 64] ->
    # per tile [128 = (w-in-pair, tk), pair]
    madd = np.where(mask == 0, np.float32(MASK_NEG), np.float32(0.0))

    wq_t = np.ascontiguousarray((Wq.T * (SCALE * WSCALE)).astype(NP_FP8))
    wk_t = np.ascontiguousarray((Wk.T * WSCALE).astype(NP_FP8))
    wv_t = np.ascontiguousarray(Wv.T.astype(NP_BF16))
    wp_t = np.ascontiguousarray(Wp.T.astype(NP_BF16))
    ones2 = np.zeros((128, 2), NP_BF16)
    ones2[0:64, 0] = 1
    ones2[64:128, 1] = 1
    wsel = np.zeros((4, 8, 128), NP_BF16)
    for ch in range(4):
        wsel[ch, 2 * ch, 0:64] = 1
        wsel[ch, 2 * ch + 1, 64:128] = 1
    ident = np.eye(128, dtype=NP_BF16)

    tok_pc = win_pc * n
    in_maps = []
    for c in range(ncores):
        sl = slice(c * tok_pc, (c + 1) * tok_pc)
        mslice = madd[c * win_pc : (c + 1) * win_pc]  # [win_pc, 64]
        mcol = np.ascontiguousarray(
            mslice.reshape(n_tt, 4, 2, 64)
            .transpose(0, 2, 3, 1)
            .reshape(n_tt, 128, 4)
        )
        in_maps.append(
            {
                "qT": _feature_major_tiles(qf[sl].astype(NP_FP8)),
                "kT": _feature_major_tiles(kf[sl].astype(NP_FP8)),
                "vT": _feature_major_tiles(vf[sl].astype(NP_BF16)),
                "maskcol": mcol,
                "wq": wq_t,
                "wk": wk_t,
                "wv": wv_t,
                "wp": wp_t,
                "ones2": ones2,
                "wsel": wsel,
                "ident": ident,
            }
        )
    return in_maps


def _reference_numpy(q, k, v, mask, Wq, bq, Wk, bk, Wv, bv, Wp, bp):
    """Full-precision host fallback (only used for nonzero bq/bk)."""
    B, n, C = q.shape
    qh = (q.reshape(-1, C) @ Wq.T + bq).reshape(B, n, HEADS, HD).transpose(0, 2, 1, 3)
    kh = (k.reshape(-1, C) @ Wk.T + bk).reshape(B, n, HEADS, HD).transpose(0, 2, 1, 3)
    vh = (v.reshape(-1, C) @ Wv.T + bv).reshape(B, n, HEADS, HD).transpose(0, 2, 1, 3)
    s = np.einsum("bhqd,bhkd->bhqk", qh * SCALE, kh)
    s = np.where((mask[:, None, None, :] == 0), np.float32(MASK_NEG), s)
    s = s - s.max(-1, keepdims=True)
    e = np.exp(s)
    p = e / e.sum(-1, keepdims=True)
    x = np.einsum("bhqk,bhkd->bhqd", p, vh)
    x = x.transpose(0, 2, 1, 3).reshape(B, n, C)
    return (x @ Wp.T + bp).astype(np.float32)


def kernel(q, k, v, mask, Wq, bq, Wk, bk, Wv, bv, Wp, bp, trace=False):
    q = np.asarray(q, np.float32)
    k = np.asarray(k, np.float32)
    v = np.asarray(v, np.float32)
    mask = np.asarray(mask)
    Wq, Wk, Wv, Wp = (np.asarray(w, np.float32) for w in (Wq, Wk, Wv, Wp))
    bq, bk, bv, bp = (np.asarray(b, np.float32) for b in (bq, bk, bv, bp))

    if np.any(bq) or np.any(bk):
        return _reference_numpy(q, k, v, mask, Wq, bq, Wk, bk, Wv, bv, Wp, bp)

    B, n, C = q.shape
    win_pc = B // NCORES
    nc = _get_program(win_pc)
    in_maps = make_in_maps(q, k, v, mask, Wq, Wk, Wv, Wp)
    res = run_bass_kernel_spmd(
        nc, in_maps, core_ids=list(range(NCORES)), trace=trace
    )
    outs = np.concatenate(
        [np.asarray(r["out"]).astype(np.float32) for r in res.results], axis=0
    )
    outs = outs.reshape(B, n, C)
    # bv flows through attention linearly (softmax rows sum to 1); with bp it
    # folds into a single output bias.
    bout = bp + bv @ Wp.T
    if np.any(bout):
        outs = outs + bout.astype(np.float32)
    if trace:
        kernel._last_result = res
    return outs


# revision 25
# speedup vs baseline: 1.3198x; 1.3198x over previous
"""Bass/Trainium2 kernel for windowed multi-head attention (8 NeuronCores).

Math per window b (64 tokens), matching torch-style nn.MultiHeadAttention:
  qh = (q @ Wq.T + bq) * scale; kh = k @ Wk.T + bk; vh = v @ Wv.T + bv
  S  = qh @ kh.T  (per head);  S[masked k] = -1e4;  P = softmax(S)
  out = concat_h(P @ vh) @ Wp.T + bp

Device dataflow (per core, 256 windows = 16384 tokens).  This revision
computes attention TRANSPOSED (S.T with key-tokens on partitions) which
removes the mask matmul, all PE transposes, the P.T ACT copy and the DVE
row-sum reduce of the previous design:
  - q,k,v pre-transposed on host to feature-major [128, 4, TT] blocks.
  - q,k and Wq,Wk are fp8(e4m3) DoubleRow (host prescale by 64; q also
    by softmax SCALE; PSUM->SBUF copies descale by 1/64).
  - k -> plain feature-major ykT; q -> zero-padded per-head-slot yqTz
    (the padding keeps the two heads of a chunk separate in the K=128
    S.T matmuls); v -> token-major yv.
  - Per window-pair: S.T[tk, (h, tq)] via 8 K=128 matmuls; mask applied
    as the per-partition bias AP of the Exp activation (tk is the
    partition axis, so the additive mask is a [128,1] vector); softmax
    denominators computed TRANSPOSED (denT[(par,tq),(ch,w)] via 4 F=2
    matmuls with the pexpT slab stationary) so the reciprocal is a
    cheap [128,8] DVE op (reciprocal runs ~8 cycles/elem/lane - a
    [2,512] layout would cost 3.3us); the reciprocals are PE-transposed
    to [8,128] and re-broadcast across window halves with 4 K=8
    selector matmuls; P.T is normalized by one DVE multiply, its odd
    half relocated to partition base 0 by a gpsimd copy, then X.T via
    16 64x64 matmuls and the output projection from X.T chunks; output
    stored bf16 (host casts to f32).
  - The whole thing is software-pipelined: a 3-stage pair pipeline
    (front = S/exp/denominators, mid = broadcast/normalize/X.T,
    back = output projection/store, emitted back-front-mid) with the
    NEXT tile's projection matmuls interleaved between pair slots so
    the PE queue never drains (an empty PE window re-throttles the HAM
    clock gate to 1.2 GHz for ~3.4us, which is why engine-idle gaps
    cost roughly double their length).
  - PSUM->SBUF drains are balanced across ACT (k copies, q even-half
    copies, exp, X.T drain) and DVE (v copies, q odd-half copies,
    reciprocal, normalize, out drain): the two drain engines together
    are the co-bottleneck with the PE (each 512-col drain costs
    ~(N+352)/1.2 ns on ACT / ~(N+130)/0.96 on DVE regardless of the
    partition count).

All matmul operands sit at partition base 0 with K in {64, 128, 256}:
mixing row-group offsets across back-to-back K<128 matmuls crashes the
PE, and DoubleRow matmuls additionally require dst partition base 0.

Biases are zero in this problem; nonzero bv/bp fold into a host-side
output add, nonzero bq/bk fall back to a host computation.
"""

import sys

for _p in ("/opt/trn_rl_repo",):
    if _p not in sys.path:
        sys.path.append(_p)

import numpy as np
import ml_dtypes

import concourse.bacc as bacc
import concourse.bass as bass
import concourse.mybir as mybir
import concourse.tile as tile
from concourse.bass import ds, ts
from concourse.bass_utils import run_bass_kernel_spmd

BF16 = mybir.dt.bfloat16
FP8 = mybir.dt.float8e4
F32 = mybir.dt.float32
NP_BF16 = ml_dtypes.bfloat16
NP_FP8 = ml_dtypes.float8_e4m3
WSCALE = 64.0  # fp8 weight prescale (host) / copy descale (device)

NCORES = 8
DIM = 512
HEADS = 8
HD = 64
N = 64  # window length
B_TOTAL = 2048
WIN_PER_CORE = B_TOTAL // NCORES  # 256
TOK_PER_CORE = WIN_PER_CORE * N  # 16384
SCALE = HD ** -0.5
MASK_NEG = -10000.0
TT = 512  # token tile (8 windows, 4 pairs)


def build_program(win_per_core=WIN_PER_CORE):
    """Build the per-core Tile program. All 8 cores run it SPMD."""
    tok = win_per_core * N
    n_tt = tok // TT
    PAIRS = TT // 128  # 4

    nc = bacc.Bacc("TRN2", debug=False)

    qT_d = nc.dram_tensor("qT", [n_tt, 128, 4, TT], FP8, kind="ExternalInput")
    kT_d = nc.dram_tensor("kT", [n_tt, 128, 4, TT], FP8, kind="ExternalInput")
    vT_d = nc.dram_tensor("vT", [n_tt, 128, 4, TT], BF16, kind="ExternalInput")
    maskcol = nc.dram_tensor(
        "maskcol", [n_tt, 128, PAIRS], F32, kind="ExternalInput"
    )
    wq = nc.dram_tensor("wq", [DIM, DIM], FP8, kind="ExternalInput")
    wk = nc.dram_tensor("wk", [DIM, DIM], FP8, kind="ExternalInput")
    wv = nc.dram_tensor("wv", [DIM, DIM], BF16, kind="ExternalInput")
    wp = nc.dram_tensor("wp", [DIM, DIM], BF16, kind="ExternalInput")
    ones2 = nc.dram_tensor("ones2", [128, 2], BF16, kind="ExternalInput")
    wsel = nc.dram_tensor("wsel", [4, 8, 128], BF16, kind="ExternalInput")
    ident = nc.dram_tensor("ident", [128, 128], BF16, kind="ExternalInput")
    out = nc.dram_tensor("out", [tok, DIM], BF16, kind="ExternalOutput")

    qa, ka, va, ma = qT_d.ap(), kT_d.ap(), vT_d.ap(), maskcol.ap()
    oa = out.ap()

    with tile.TileContext(nc) as tc:
        with (
            tc.tile_pool(name="consts", bufs=1) as consts,
            tc.tile_pool(name="pin", bufs=3) as pin,
            tc.tile_pool(name="py", bufs=3) as py,
            tc.tile_pool(name="pp", bufs=6) as pp,
            tc.tile_pool(name="pout", bufs=4) as pout,
            tc.tile_pool(name="ps_pj", bufs=3, space="PSUM") as ps_pj,
            tc.tile_pool(name="ps_s", bufs=2, space="PSUM") as ps_s,
            tc.tile_pool(name="ps_x", bufs=2, space="PSUM") as ps_x,
            tc.tile_pool(name="ps_o", bufs=1, space="PSUM") as ps_o,
        ):
            # Weights: [fi_part 128, fi_chunk 4, fo 512]
            w_tiles = {}
            for name, t in (("wq", wq), ("wk", wk), ("wv", wv), ("wp", wp)):
                dt_ = FP8 if name in ("wq", "wk") else BF16
                wt = consts.tile([128, 4, DIM], dt_, tag=f"w_{name}")
                nc.sync.dma_start(
                    out=wt, in_=t.ap().rearrange("(c p) f -> p c f", p=128)
                )
                w_tiles[name] = wt
            # ones2: col w sums window w's partitions; sel2: row w
            # broadcasts to window w's partition half
            ones_t = consts.tile([128, 2], BF16, tag="ones2")
            nc.sync.dma_start(out=ones_t, in_=ones2.ap())
            # wsel[ch][(ch2, w), tkg] = (ch2 == ch) & (tkg // 64 == w):
            # selects this chunk's reciprocal row for each window half
            wsel_t = []
            for ch in range(4):
                wt_ = consts.tile([8, 128], BF16, tag=f"wsel{ch}", name=f"wsel{ch}")
                nc.sync.dma_start(out=wt_, in_=wsel.ap()[ch])
                wsel_t.append(wt_)
            id8_t = consts.tile([128, 128], BF16, tag="id8")
            nc.sync.dma_start(out=id8_t, in_=ident.ap())

            # Two persistent zero-padded q tiles, manually alternated per
            # T-tile: their padding slots are zeroed exactly once and are
            # never overwritten afterwards.
            yqTz_bufs = []
            for i in range(2):
                bz = consts.tile(
                    [128, 4, 2, TT], BF16, tag=f"yqTz{i}", name=f"yqTz{i}"
                )
                nc.gpsimd.memset(bz[0:64, :, 1, :], 0.0)
                nc.gpsimd.memset(bz[64:128, :, 0, :], 0.0)
                yqTz_bufs.append(bz)

            def emit_inputs(t):
                qT = pin.tile([128, 4, TT], FP8, tag="qT", name="qT")
                kT = pin.tile([128, 4, TT], FP8, tag="kT", name="kT")
                vT = pin.tile([128, 4, TT], BF16, tag="vT", name="vT")
                mc = pin.tile([128, PAIRS], F32, tag="mc", name="mc")
                nc.sync.dma_start(out=qT, in_=qa[t])
                nc.sync.dma_start(out=kT, in_=ka[t])
                nc.sync.dma_start(out=vT, in_=va[t])
                nc.sync.dma_start(out=mc, in_=ma[t])
                return qT, kT, vT, mc

            def emit_kproj(kT):
                # k projection -> plain feature-major bf16 [fo 128, c, tok]
                ykT = py.tile([128, 4, TT], BF16, tag="ykT", name="ykT")
                for co in range(4):
                    ps = ps_pj.tile([128, TT], F32, tag="pj", name="ps")
                    for i in range(2):
                        nc.tensor.matmul(
                            ps,
                            lhsT=w_tiles["wk"][:, 2 * i : 2 * i + 2, ts(co, 128)],
                            rhs=kT[:, 2 * i : 2 * i + 2, :],
                            start=(i == 0),
                            stop=(i == 1),
                            perf_mode=mybir.MatmulPerfMode.DoubleRow,
                        )
                    nc.scalar.activation(
                        out=ykT[:, co, :],
                        in_=ps,
                        func=mybir.ActivationFunctionType.Copy,
                        scale=1.0 / WSCALE,
                    )
                return ykT

            def emit_qproj(qT, t):
                # q projection -> zero-padded per-head-slot layout:
                # yqTz[0:64, c, 0, :] = head 2c rows, yqTz[64:128, c, 1, :]
                # = head 2c+1 rows, other slots stay zero so the K=128 S.T
                # matmuls keep the two heads of a chunk separate.  The two
                # PSUM half drains go to different engines so the bank
                # frees at the speed of one drain, not two.
                yqTz = yqTz_bufs[t % 2]
                for co in range(4):
                    ps = ps_pj.tile([128, TT], F32, tag="pj", name="ps")
                    for i in range(2):
                        nc.tensor.matmul(
                            ps,
                            lhsT=w_tiles["wq"][:, 2 * i : 2 * i + 2, ts(co, 128)],
                            rhs=qT[:, 2 * i : 2 * i + 2, :],
                            start=(i == 0),
                            stop=(i == 1),
                            perf_mode=mybir.MatmulPerfMode.DoubleRow,
                        )
                    nc.scalar.activation(
                        out=yqTz[0:64, co, 0, :],
                        in_=ps[0:64, :],
                        func=mybir.ActivationFunctionType.Copy,
                        scale=1.0 / WSCALE,
                    )
                    nc.vector.tensor_scalar_mul(
                        yqTz[64:128, co, 1, :], ps[64:128, :], 1.0 / WSCALE
                    )
                return yqTz

            def emit_vproj_half(vT, yv, cts):
                # v projection -> token-major bf16 [tok%128, pair, fo];
                # emitted in two halves on different pair slots to even
                # out the PE load per slot
                if yv is None:
                    yv = py.tile([128, PAIRS, DIM], BF16, tag="yv", name="yv")
                for ct in cts:
                    ps = ps_pj.tile([128, DIM], F32, tag="pj", name="ps")
                    for ci in range(4):
                        nc.tensor.matmul(
                            ps,
                            lhsT=vT[:, ci, ts(ct, 128)],
                            rhs=w_tiles["wv"][:, ci, :],
                            start=(ci == 0),
                            stop=(ci == 3),
                        )
                    if ct % 2 == 0:
                        nc.scalar.copy(out=yv[:, ct, :], in_=ps)
                    else:
                        nc.vector.tensor_copy(out=yv[:, ct, :], in_=ps)
                return yv

            def emit_vhi(yv):
                # second window's vh rows relocated to partition base 0
                yv_hi = py.tile([64, PAIRS, DIM], BF16, tag="yv_hi", name="yv_hi")
                nc.gpsimd.dma_start(out=yv_hi, in_=yv[64:128, :, :])
                return yv_hi

            # ---- prologue: tile 0 inputs + projections ----
            qT0, kT0, vT0, mc0 = emit_inputs(0)
            yv0 = emit_vproj_half(vT0, None, range(PAIRS))
            proj = {
                0: (
                    emit_kproj(kT0),
                    emit_qproj(qT0, 0),
                    yv0,
                    emit_vhi(yv0),
                    mc0,
                )
            }
            pend_in = None

            # ---- attention pipeline over all (tile, pair) slots ----
            # Three-stage software pipeline (one extra pair of latency each):
            #   front(s):  S.T matmuls, exp, transposed denominators,
            #              reciprocal, PE transpose + [8, 128] drain
            #   mid(s-1):  reciprocal broadcast, normalize P.T, X.T
            #              matmuls, X.T drain
            #   back(s-2): output projection, out drain, store
            # The NEXT tile's projection matmuls + drains are interleaved
            # between pair slots so the PE never drains its queue while the
            # ACT engine catches up (an empty PE window re-throttles the
            # HAM clock gate to 1.2 GHz for ~3.4 us).
            st1 = st2 = None
            total = n_tt * PAIRS + 2
            for s in range(total):
                t, pr = divmod(s, PAIRS)
                if st2 is not None:
                    xts_b, ab2 = st2
                    # output projection: [tok 128, fo 512]
                    po = ps_o.tile([128, DIM], F32, tag="po", name="po")
                    for c in range(4):
                        nc.tensor.matmul(
                            po,
                            lhsT=xts_b[:, c, :],
                            rhs=w_tiles["wp"][:, c, :],
                            start=(c == 0),
                            stop=(c == 3),
                        )
                    osb = pout.tile([128, DIM], BF16, tag="osb", name="osb")
                    nc.vector.tensor_copy(out=osb, in_=po)
                    nc.sync.dma_start(out=oa[ab2 : ab2 + 128, :], in_=osb)
                nst1 = None
                if t < n_tt:
                    ykT, yqTz, yv, yv_hi, mc = proj[t]
                    base = pr * 128
                    # S.T: [(w, tk) 128, (head, tq) 512].  Each (ch, w)
                    # region is written exactly once -> every matmul is its
                    # own accumulation group (start only clears has_written
                    # for the region it writes).
                    spT = ps_s.tile([128, HEADS, N], F32, tag="spT", name="spT")
                    for ch in range(4):
                        for w in range(2):
                            nc.tensor.matmul(
                                spT[ds(64 * w, 64), ds(2 * ch, 2), :],
                                lhsT=ykT[:, ch, ds(base + 64 * w, 64)],
                                rhs=yqTz[:, ch, :, ds(base + 64 * w, 64)],
                                start=True,
                                stop=True,
                                skip_group_check=True,
                            )
                    # softmax numerator: exp(S.T + mask[tk]); the additive
                    # mask rides the ACT's per-partition bias port.
                    pexpT = pp.tile([128, HEADS, N], BF16, tag="pexpT", name="pexpT")
                    nc.scalar.activation(
                        out=pexpT,
                        in_=spT,
                        func=mybir.ActivationFunctionType.Exp,
                        bias=mc[:, pr : pr + 1],
                    )
                    # transposed denominators denT[(par, tq) 128, (ch, w)]
                    # + the reciprocal transpose share one scratch PSUM bank
                    # (bitcast views), so spT's bank frees right after exp
                    # and the next slot's S.T matmuls are not chained behind
                    # the reciprocal.  This orientation makes the reciprocal
                    # a cheap [128, 8] op (~8 DVE cycles per element/lane).
                    scr = ps_x.tile([128, 1024], BF16, tag="xt", name="scr")
                    denT8 = scr[:, 0:16].bitcast(F32)
                    denT = denT8.rearrange("p (c w) -> p c w", c=4)
                    for ch in range(4):
                        nc.tensor.matmul(
                            denT[:, ch, :],
                            lhsT=pexpT[:, 2 * ch : 2 * ch + 2, :],
                            rhs=ones_t,
                            start=True,
                            stop=True,
                            skip_group_check=True,
                        )
                    rT = pp.tile([128, 8], BF16, tag="rT", name="rT")
                    with nc.allow_low_precision(
                        reason="softmax denominators are O(1..64) sums; "
                        "bf16 reciprocal is inside the fp8 error budget"
                    ):
                        nc.vector.reciprocal(out=rT, in_=denT8)
                    # PE-transpose the [128, 8] reciprocals to [8, 128]
                    # (rows become (ch, w)), then a cheap [8, 128] drain
                    recT_ps = scr[0:8, 128:256]
                    nc.tensor.transpose(recT_ps, rT, id8_t)
                    rec2T = pp.tile([8, 128], BF16, tag="rec2T", name="rec2T")
                    nc.vector.tensor_copy(out=rec2T, in_=recT_ps)
                    nst1 = (pexpT, rec2T, yv, yv_hi, t * TT + base)
                if st1 is not None:
                    pexpT_b, rec2T_b, yv_b, yv_hi_b, ab = st1
                    prb = (ab % TT) // 128
                    # broadcast reciprocals across the window halves:
                    # rec_bc[tkg, ch, par, tq] = rec2T[(ch, w(tkg)), par, tq]
                    rec_bc = ps_s.tile(
                        [128, 4, 2, N], F32, tag="spT", name="rec_bc"
                    )
                    for ch in range(4):
                        nc.tensor.matmul(
                            rec_bc[:, ch, :, :],
                            lhsT=wsel_t[ch],
                            rhs=rec2T_b,
                            start=True,
                            stop=True,
                            skip_group_check=True,
                        )
                    # normalized P.T (in1 reads the PSUM broadcast directly)
                    pnT = pp.tile([128, HEADS, N], BF16, tag="pnT", name="pnT")
                    nc.vector.tensor_tensor(
                        out=pnT,
                        in0=pexpT_b,
                        in1=rec_bc.rearrange("p c w n -> p (c w) n"),
                        op=mybir.AluOpType.mult,
                    )
                    # odd-window rows relocated to partition base 0 (the
                    # gpsimd DMA path: the HWDGE queues are congested with
                    # input/output traffic while GpSimd is idle)
                    pnT_hi = pp.tile([64, HEADS, N], BF16, tag="pnT_hi", name="pnT_hi")
                    nc.gpsimd.dma_start(out=pnT_hi, in_=pnT[64:128, :, :])
                    # X.T: [feat-in-chunk 128, chunk 4, (w, tq) 128]
                    xt = ps_x.tile([128, 4, 128], F32, tag="xt", name="xt")
                    for h in range(HEADS):
                        ch, hh = h // 2, (h % 2) * 64
                        for w in range(2):
                            psrc = pnT if w == 0 else pnT_hi
                            vsrc = yv_b if w == 0 else yv_hi_b
                            nc.tensor.matmul(
                                xt[ds(hh, 64), ch, ds(64 * w, 64)],
                                lhsT=vsrc[ds(0, 64), prb, ts(h, 64)],
                                rhs=psrc[ds(0, 64), h, :],
                                start=True,
                                stop=True,
                            )
                    xts = pp.tile([128, 4, 128], BF16, tag="xts", name="xts")
                    nc.scalar.copy(out=xts, in_=xt)
                    st2 = (xts, ab)
                else:
                    st2 = None
                st1 = nst1
                # interleave the next tile's inputs + projections between
                # this tile's pair slots
                if t + 1 < n_tt:
                    if pr == 0:
                        pend_in = emit_inputs(t + 1)
                        pend_k = emit_kproj(pend_in[1])
                    elif pr == 1:
                        pend_q = emit_qproj(pend_in[0], t + 1)
                    elif pr == 2:
                        pend_v = emit_vproj_half(pend_in[2], None, range(PAIRS))
                    elif pr == 3:
                        proj[t + 1] = (
                            pend_k,
                            pend_q,
                            pend_v,
                            emit_vhi(pend_v),
                            pend_in[3],
                        )
                        del proj[t]

    nc.compile()
    return nc


_PROGRAM_CACHE = {}


def _get_program(win_per_core):
    if win_per_core not in _PROGRAM_CACHE:
        _PROGRAM_CACHE[win_per_core] = build_program(win_per_core)
    return _PROGRAM_CACHE[win_per_core]


def _feature_major_tiles(x_flat_bf16):
    """[tok, 512] -> [n_tt, 128, 4, TT] so each T-tile block is one
    fully-contiguous transposed DMA read."""
    tok = x_flat_bf16.shape[0]
    n_tt = tok // TT
    xt = x_flat_bf16.reshape(n_tt, TT, 4, 128).transpose(0, 3, 2, 1)
    return np.ascontiguousarray(xt)


def make_in_maps(q, k, v, mask, Wq, Wk, Wv, Wp, ncores=NCORES):
    """Host-side shard + layout prep. Returns list of per-core input dicts."""
    B, n, C = q.shape
    win_pc = B // ncores
    n_tt = win_pc * n // TT
    qf = np.ascontiguousarray(q.reshape(B * n, C))
    kf = np.ascontiguousarray(k.reshape(B * n, C))
    vf = np.ascontiguousarray(v.reshape(B * n, C))

    # additive mask as a per-key-token column: [B(win), 64] ->
    # per tile [128 = (w-in-pair, tk), pair]
    madd = np.where(mask == 0, np.float32(MASK_NEG), np.float32(0.0))

    wq_t = np.ascontiguousarray((Wq.T * (SCALE * WSCALE)).astype(NP_FP8))
    wk_t = np.ascontiguousarray((Wk.T * WSCALE).astype(NP_FP8))
    wv_t = np.ascontiguousarray(Wv.T.astype(NP_BF16))
    wp_t = np.ascontiguousarray(Wp.T.astype(NP_BF16))
    ones2 = np.zeros((128, 2), NP_BF16)
    ones2[0:64, 0] = 1
    ones2[64:128, 1] = 1
    wsel = np.zeros((4, 8, 128), NP_BF16)
    for ch in range(4):
        wsel[ch, 2 * ch, 0:64] = 1
        wsel[ch, 2 * ch + 1, 64:128] = 1
    ident = np.eye(128, dtype=NP_BF16)

    tok_pc = win_pc * n
    in_maps = []
    for c in range(ncores):
        sl = slice(c * tok_pc, (c + 1) * tok_pc)
        mslice = madd[c * win_pc : (c + 1) * win_pc]  # [win_pc, 64]
        mcol = np.ascontiguousarray(
            mslice.reshape(n_tt, 4, 2, 64)
            .transpose(0, 2, 3, 1)
            .reshape(n_tt, 128, 4)
        )
        in_maps.append(
            {
                "qT": _feature_major_tiles(qf[sl].astype(NP_FP8)),
                "kT": _feature_major_tiles(kf[sl].astype(NP_FP8)),
                "vT": _feature_major_tiles(vf[sl].astype(NP_BF16)),
                "maskcol": mcol,
                "wq": wq_t,
                "wk": wk_t,
                "wv": wv_t,
                "wp": wp_t,
                "ones2": ones2,
                "wsel": wsel,
                "ident": ident,
            }
        )
    return in_maps


def _reference_numpy(q, k, v, mask, Wq, bq, Wk, bk, Wv, bv, Wp, bp):
    """Full-precision host fallback (only used for nonzero bq/bk)."""
    B, n, C = q.shape
    qh = (q.reshape(-1, C) @ Wq.T + bq).reshape(B, n, HEADS, HD).transpose(0, 2, 1, 3)
    kh = (k.reshape(-1, C) @ Wk.T + bk).reshape(B, n, HEADS, HD).transpose(0, 2, 1, 3)
    vh = (v.reshape(-1, C) @ Wv.T + bv).reshape(B, n, HEADS, HD).transpose(0, 2, 1, 3)
    s = np.einsum("bhqd,bhkd->bhqk", qh * SCALE, kh)
    s = np.where((mask[:, None, None, :] == 0), np.float32(MASK_NEG), s)
    s = s - s.max(-1, keepdims=True)
    e = np.exp(s)
    p = e / e.sum(-1, keepdims=True)
    x = np.einsum("bhqk,bhkd->bhqd", p, vh)
    x = x.transpose(0, 2, 1, 3).reshape(B, n, C)
    return (x @ Wp.T + bp).astype(np.float32)


def kernel(q, k, v, mask, Wq, bq, Wk, bk, Wv, bv, Wp, bp, trace=False):
    q = np.asarray(q, np.float32)
    k = np.asarray(k, np.float32)
    v = np.asarray(v, np.float32)
    mask = np.asarray(mask)
    Wq, Wk, Wv, Wp = (np.asarray(w, np.float32) for w in (Wq, Wk, Wv, Wp))
    bq, bk, bv, bp = (np.asarray(b, np.float32) for b in (bq, bk, bv, bp))

    if np.any(bq) or np.any(bk):
        return _reference_numpy(q, k, v, mask, Wq, bq, Wk, bk, Wv, bv, Wp, bp)

    B, n, C = q.shape
    win_pc = B // NCORES
    nc = _get_program(win_pc)
    in_maps = make_in_maps(q, k, v, mask, Wq, Wk, Wv, Wp)
    res = run_bass_kernel_spmd(
        nc, in_maps, core_ids=list(range(NCORES)), trace=trace
    )
    outs = np.concatenate(
        [np.asarray(r["out"]).astype(np.float32) for r in res.results], axis=0
    )
    outs = outs.reshape(B, n, C)
    # bv flows through attention linearly (softmax rows sum to 1); with bp it
    # folds into a single output bias.
    bout = bp + bv @ Wp.T
    if np.any(bout):
        outs = outs + bout.astype(np.float32)
    if trace:
        kernel._last_result = res
    return outs


# revision 26
# speedup vs baseline: 1.3272x; 1.0056x over previous
"""Bass/Trainium2 kernel for windowed multi-head attention (8 NeuronCores).

Math per window b (64 tokens), matching torch-style nn.MultiHeadAttention:
  qh = (q @ Wq.T + bq) * scale; kh = k @ Wk.T + bk; vh = v @ Wv.T + bv
  S  = qh @ kh.T  (per head);  S[masked k] = -1e4;  P = softmax(S)
  out = concat_h(P @ vh) @ Wp.T + bp

Device dataflow (per core, 256 windows = 16384 tokens).  This revision
computes attention TRANSPOSED (S.T with key-tokens on partitions) which
removes the mask matmul, all PE transposes, the P.T ACT copy and the DVE
row-sum reduce of the previous design:
  - q,k,v pre-transposed on host to feature-major [128, 4, TT] blocks.
  - q,k and Wq,Wk are fp8(e4m3) DoubleRow (host prescale by 64; q also
    by softmax SCALE; PSUM->SBUF copies descale by 1/64).
  - k -> plain feature-major ykT; q -> zero-padded per-head-slot yqTz
    (the padding keeps the two heads of a chunk separate in the K=128
    S.T matmuls); v -> token-major yv.
  - Per window-pair: S.T[tk, (h, tq)] via 8 K=128 matmuls; mask applied
    as the per-partition bias AP of the Exp activation (tk is the
    partition axis, so the additive mask is a [128,1] vector); softmax
    denominators computed TRANSPOSED (denT[(par,tq),(ch,w)] via 4 F=2
    matmuls with the pexpT slab stationary) so the reciprocal is a
    cheap [128,8] DVE op (reciprocal runs ~8 cycles/elem/lane - a
    [2,512] layout would cost 3.3us); the reciprocals are PE-transposed
    to [8,128] and re-broadcast across window halves with 4 K=8
    selector matmuls; P.T is normalized by one DVE multiply, its odd
    half relocated to partition base 0 by a gpsimd copy, then X.T via
    16 64x64 matmuls and the output projection from X.T chunks; output
    stored bf16 (host casts to f32).
  - The whole thing is software-pipelined: a 3-stage pair pipeline
    (front = S/exp/denominators, mid = broadcast/normalize/X.T,
    back = output projection/store, emitted back-front-mid) with the
    NEXT tile's projection matmuls interleaved between pair slots so
    the PE queue never drains (an empty PE window re-throttles the HAM
    clock gate to 1.2 GHz for ~3.4us, which is why engine-idle gaps
    cost roughly double their length).
  - PSUM->SBUF drains are balanced across ACT (k copies, q even-half
    copies, exp, X.T drain) and DVE (v copies, q odd-half copies,
    reciprocal, normalize, out drain): the two drain engines together
    are the co-bottleneck with the PE (each 512-col drain costs
    ~(N+352)/1.2 ns on ACT / ~(N+130)/0.96 on DVE regardless of the
    partition count).

All matmul operands sit at partition base 0 with K in {64, 128, 256}:
mixing row-group offsets across back-to-back K<128 matmuls crashes the
PE, and DoubleRow matmuls additionally require dst partition base 0.

Biases are zero in this problem; nonzero bv/bp fold into a host-side
output add, nonzero bq/bk fall back to a host computation.
"""

import sys

for _p in ("/opt/trn_rl_repo",):
    if _p not in sys.path:
        sys.path.append(_p)

import numpy as np
import ml_dtypes

import concourse.bacc as bacc
import concourse.bass as bass
import concourse.mybir as mybir
import concourse.tile as tile
from concourse.bass import ds, ts
from concourse.bass_utils import run_bass_kernel_spmd

BF16 = mybir.dt.bfloat16
FP8 = mybir.dt.float8e4
F32 = mybir.dt.float32
NP_BF16 = ml_dtypes.bfloat16
NP_FP8 = ml_dtypes.float8_e4m3
WSCALE = 64.0  # fp8 weight prescale (host) / copy descale (device)

NCORES = 8
DIM = 512
HEADS = 8
HD = 64
N = 64  # window length
B_TOTAL = 2048
WIN_PER_CORE = B_TOTAL // NCORES  # 256
TOK_PER_CORE = WIN_PER_CORE * N  # 16384
SCALE = HD ** -0.5
MASK_NEG = -10000.0
TT = 512  # token tile (8 windows, 4 pairs)


def build_program(win_per_core=WIN_PER_CORE):
    """Build the per-core Tile program. All 8 cores run it SPMD."""
    tok = win_per_core * N
    n_tt = tok // TT
    PAIRS = TT // 128  # 4

    nc = bacc.Bacc("TRN2", debug=False)

    qT_d = nc.dram_tensor("qT", [n_tt, 128, 4, TT], FP8, kind="ExternalInput")
    kT_d = nc.dram_tensor("kT", [n_tt, 128, 4, TT], FP8, kind="ExternalInput")
    vT_d = nc.dram_tensor("vT", [n_tt, 128, 4, TT], BF16, kind="ExternalInput")
    maskcol = nc.dram_tensor(
        "maskcol", [n_tt, 128, PAIRS], F32, kind="ExternalInput"
    )
    wq = nc.dram_tensor("wq", [DIM, DIM], FP8, kind="ExternalInput")
    wk = nc.dram_tensor("wk", [DIM, DIM], FP8, kind="ExternalInput")
    wv = nc.dram_tensor("wv", [DIM, DIM], BF16, kind="ExternalInput")
    wp = nc.dram_tensor("wp", [DIM, DIM], BF16, kind="ExternalInput")
    ones2 = nc.dram_tensor("ones2", [128, 2], BF16, kind="ExternalInput")
    wsel = nc.dram_tensor("wsel", [4, 8, 128], BF16, kind="ExternalInput")
    ident = nc.dram_tensor("ident", [128, 128], BF16, kind="ExternalInput")
    out = nc.dram_tensor("out", [tok, DIM], BF16, kind="ExternalOutput")

    qa, ka, va, ma = qT_d.ap(), kT_d.ap(), vT_d.ap(), maskcol.ap()
    oa = out.ap()

    with tile.TileContext(nc) as tc:
        with (
            tc.tile_pool(name="consts", bufs=1) as consts,
            tc.tile_pool(name="pin", bufs=4) as pin,
            tc.tile_pool(name="py", bufs=4) as py,
            tc.tile_pool(name="pp", bufs=8) as pp,
            tc.tile_pool(name="pout", bufs=6) as pout,
            tc.tile_pool(name="ps_pj", bufs=3, space="PSUM") as ps_pj,
            tc.tile_pool(name="ps_s", bufs=2, space="PSUM") as ps_s,
            tc.tile_pool(name="ps_x", bufs=2, space="PSUM") as ps_x,
            tc.tile_pool(name="ps_o", bufs=1, space="PSUM") as ps_o,
        ):
            # Weights: [fi_part 128, fi_chunk 4, fo 512]
            w_tiles = {}
            for name, t in (("wq", wq), ("wk", wk), ("wv", wv), ("wp", wp)):
                dt_ = FP8 if name in ("wq", "wk") else BF16
                wt = consts.tile([128, 4, DIM], dt_, tag=f"w_{name}")
                nc.sync.dma_start(
                    out=wt, in_=t.ap().rearrange("(c p) f -> p c f", p=128)
                )
                w_tiles[name] = wt
            # ones2: col w sums window w's partitions; sel2: row w
            # broadcasts to window w's partition half
            ones_t = consts.tile([128, 2], BF16, tag="ones2")
            nc.sync.dma_start(out=ones_t, in_=ones2.ap())
            # wsel[ch][(ch2, w), tkg] = (ch2 == ch) & (tkg // 64 == w):
            # selects this chunk's reciprocal row for each window half
            wsel_t = []
            for ch in range(4):
                wt_ = consts.tile([8, 128], BF16, tag=f"wsel{ch}", name=f"wsel{ch}")
                nc.sync.dma_start(out=wt_, in_=wsel.ap()[ch])
                wsel_t.append(wt_)
            id8_t = consts.tile([128, 128], BF16, tag="id8")
            nc.sync.dma_start(out=id8_t, in_=ident.ap())

            # Two persistent zero-padded q tiles, manually alternated per
            # T-tile: their padding slots are zeroed exactly once and are
            # never overwritten afterwards.
            yqTz_bufs = []
            for i in range(2):
                bz = consts.tile(
                    [128, 4, 2, TT], BF16, tag=f"yqTz{i}", name=f"yqTz{i}"
                )
                nc.gpsimd.memset(bz[0:64, :, 1, :], 0.0)
                nc.gpsimd.memset(bz[64:128, :, 0, :], 0.0)
                yqTz_bufs.append(bz)

            def emit_inputs(t):
                qT = pin.tile([128, 4, TT], FP8, tag="qT", name="qT")
                kT = pin.tile([128, 4, TT], FP8, tag="kT", name="kT")
                vT = pin.tile([128, 4, TT], BF16, tag="vT", name="vT")
                mc = pin.tile([128, PAIRS], F32, tag="mc", name="mc")
                nc.sync.dma_start(out=qT, in_=qa[t])
                nc.sync.dma_start(out=kT, in_=ka[t])
                nc.sync.dma_start(out=vT, in_=va[t])
                nc.sync.dma_start(out=mc, in_=ma[t])
                return qT, kT, vT, mc

            def emit_kproj(kT):
                # k projection -> plain feature-major bf16 [fo 128, c, tok]
                ykT = py.tile([128, 4, TT], BF16, tag="ykT", name="ykT")
                for co in range(4):
                    ps = ps_pj.tile([128, TT], F32, tag="pj", name="ps")
                    for i in range(2):
                        nc.tensor.matmul(
                            ps,
                            lhsT=w_tiles["wk"][:, 2 * i : 2 * i + 2, ts(co, 128)],
                            rhs=kT[:, 2 * i : 2 * i + 2, :],
                            start=(i == 0),
                            stop=(i == 1),
                            perf_mode=mybir.MatmulPerfMode.DoubleRow,
                        )
                    nc.scalar.activation(
                        out=ykT[:, co, :],
                        in_=ps,
                        func=mybir.ActivationFunctionType.Copy,
                        scale=1.0 / WSCALE,
                    )
                return ykT

            def emit_qproj(qT, t):
                # q projection -> zero-padded per-head-slot layout:
                # yqTz[0:64, c, 0, :] = head 2c rows, yqTz[64:128, c, 1, :]
                # = head 2c+1 rows, other slots stay zero so the K=128 S.T
                # matmuls keep the two heads of a chunk separate.  The two
                # PSUM half drains go to different engines so the bank
                # frees at the speed of one drain, not two.
                yqTz = yqTz_bufs[t % 2]
                for co in range(4):
                    ps = ps_pj.tile([128, TT], F32, tag="pj", name="ps")
                    for i in range(2):
                        nc.tensor.matmul(
                            ps,
                            lhsT=w_tiles["wq"][:, 2 * i : 2 * i + 2, ts(co, 128)],
                            rhs=qT[:, 2 * i : 2 * i + 2, :],
                            start=(i == 0),
                            stop=(i == 1),
                            perf_mode=mybir.MatmulPerfMode.DoubleRow,
                        )
                    nc.scalar.activation(
                        out=yqTz[0:64, co, 0, :],
                        in_=ps[0:64, :],
                        func=mybir.ActivationFunctionType.Copy,
                        scale=1.0 / WSCALE,
                    )
                    nc.vector.tensor_scalar_mul(
                        yqTz[64:128, co, 1, :], ps[64:128, :], 1.0 / WSCALE
                    )
                return yqTz

            def emit_vproj_half(vT, yv, cts):
                # v projection -> token-major bf16 [tok%128, pair, fo];
                # emitted in two halves on different pair slots to even
                # out the PE load per slot
                if yv is None:
                    yv = py.tile([128, PAIRS, DIM], BF16, tag="yv", name="yv")
                for ct in cts:
                    ps = ps_pj.tile([128, DIM], F32, tag="pj", name="ps")
                    for ci in range(4):
                        nc.tensor.matmul(
                            ps,
                            lhsT=vT[:, ci, ts(ct, 128)],
                            rhs=w_tiles["wv"][:, ci, :],
                            start=(ci == 0),
                            stop=(ci == 3),
                        )
                    if ct % 2 == 0:
                        nc.scalar.copy(out=yv[:, ct, :], in_=ps)
                    else:
                        nc.vector.tensor_copy(out=yv[:, ct, :], in_=ps)
                return yv

            def emit_vhi(yv):
                # second window's vh rows relocated to partition base 0
                yv_hi = py.tile([64, PAIRS, DIM], BF16, tag="yv_hi", name="yv_hi")
                nc.gpsimd.dma_start(out=yv_hi, in_=yv[64:128, :, :])
                return yv_hi

            # ---- prologue: tile 0 inputs + projections ----
            qT0, kT0, vT0, mc0 = emit_inputs(0)
            yv0 = emit_vproj_half(vT0, None, range(PAIRS))
            proj = {
                0: (
                    emit_kproj(kT0),
                    emit_qproj(qT0, 0),
                    yv0,
                    emit_vhi(yv0),
                    mc0,
                )
            }
            pend_in = None

            # ---- attention pipeline over all (tile, pair) slots ----
            # Three-stage software pipeline (one extra pair of latency each):
            #   front(s):  S.T matmuls, exp, transposed denominators,
            #              reciprocal, PE transpose + [8, 128] drain
            #   mid(s-1):  reciprocal broadcast, normalize P.T, X.T
            #              matmuls, X.T drain
            #   back(s-2): output projection, out drain, store
            # The NEXT tile's projection matmuls + drains are interleaved
            # between pair slots so the PE never drains its queue while the
            # ACT engine catches up (an empty PE window re-throttles the
            # HAM clock gate to 1.2 GHz for ~3.4 us).
            st1 = st2 = None
            total = n_tt * PAIRS + 2
            for s in range(total):
                t, pr = divmod(s, PAIRS)
                if st2 is not None:
                    xts_b, ab2 = st2
                    # output projection: [tok 128, fo 512]
                    po = ps_o.tile([128, DIM], F32, tag="po", name="po")
                    for c in range(4):
                        nc.tensor.matmul(
                            po,
                            lhsT=xts_b[:, c, :],
                            rhs=w_tiles["wp"][:, c, :],
                            start=(c == 0),
                            stop=(c == 3),
                        )
                    osb = pout.tile([128, DIM], BF16, tag="osb", name="osb")
                    nc.vector.tensor_copy(out=osb, in_=po)
                    nc.sync.dma_start(out=oa[ab2 : ab2 + 128, :], in_=osb)
                nst1 = None
                if t < n_tt:
                    ykT, yqTz, yv, yv_hi, mc = proj[t]
                    base = pr * 128
                    # S.T: [(w, tk) 128, (head, tq) 512].  Each (ch, w)
                    # region is written exactly once -> every matmul is its
                    # own accumulation group (start only clears has_written
                    # for the region it writes).
                    spT = ps_s.tile([128, HEADS, N], F32, tag="spT", name="spT")
                    for ch in range(4):
                        for w in range(2):
                            nc.tensor.matmul(
                                spT[ds(64 * w, 64), ds(2 * ch, 2), :],
                                lhsT=ykT[:, ch, ds(base + 64 * w, 64)],
                                rhs=yqTz[:, ch, :, ds(base + 64 * w, 64)],
                                start=True,
                                stop=True,
                                skip_group_check=True,
                            )
                    # softmax numerator: exp(S.T + mask[tk]); the additive
                    # mask rides the ACT's per-partition bias port.
                    pexpT = pp.tile([128, HEADS, N], BF16, tag="pexpT", name="pexpT")
                    nc.scalar.activation(
                        out=pexpT,
                        in_=spT,
                        func=mybir.ActivationFunctionType.Exp,
                        bias=mc[:, pr : pr + 1],
                    )
                    # transposed denominators denT[(par, tq) 128, (ch, w)]
                    # + the reciprocal transpose share one scratch PSUM bank
                    # (bitcast views), so spT's bank frees right after exp
                    # and the next slot's S.T matmuls are not chained behind
                    # the reciprocal.  This orientation makes the reciprocal
                    # a cheap [128, 8] op (~8 DVE cycles per element/lane).
                    scr = ps_x.tile([128, 1024], BF16, tag="xt", name="scr")
                    denT8 = scr[:, 0:16].bitcast(F32)
                    denT = denT8.rearrange("p (c w) -> p c w", c=4)
                    for ch in range(4):
                        nc.tensor.matmul(
                            denT[:, ch, :],
                            lhsT=pexpT[:, 2 * ch : 2 * ch + 2, :],
                            rhs=ones_t,
                            start=True,
                            stop=True,
                            skip_group_check=True,
                        )
                    rT = pp.tile([128, 8], BF16, tag="rT", name="rT")
                    with nc.allow_low_precision(
                        reason="softmax denominators are O(1..64) sums; "
                        "bf16 reciprocal is inside the fp8 error budget"
                    ):
                        nc.vector.reciprocal(out=rT, in_=denT8)
                    # PE-transpose the [128, 8] reciprocals to [8, 128]
                    # (rows become (ch, w)), then a cheap [8, 128] drain
                    recT_ps = scr[0:8, 128:256]
                    nc.tensor.transpose(recT_ps, rT, id8_t)
                    rec2T = pp.tile([8, 128], BF16, tag="rec2T", name="rec2T")
                    nc.vector.tensor_copy(out=rec2T, in_=recT_ps)
                    nst1 = (pexpT, rec2T, yv, yv_hi, t * TT + base)
                if st1 is not None:
                    pexpT_b, rec2T_b, yv_b, yv_hi_b, ab = st1
                    prb = (ab % TT) // 128
                    # broadcast reciprocals across the window halves:
                    # rec_bc[tkg, ch, par, tq] = rec2T[(ch, w(tkg)), par, tq]
                    rec_bc = ps_s.tile(
                        [128, 4, 2, N], F32, tag="spT", name="rec_bc"
                    )
                    for ch in range(4):
                        nc.tensor.matmul(
                            rec_bc[:, ch, :, :],
                            lhsT=wsel_t[ch],
                            rhs=rec2T_b,
                            start=True,
                            stop=True,
                            skip_group_check=True,
                        )
                    # normalized P.T (in1 reads the PSUM broadcast directly)
                    pnT = pp.tile([128, HEADS, N], BF16, tag="pnT", name="pnT")
                    nc.vector.tensor_tensor(
                        out=pnT,
                        in0=pexpT_b,
                        in1=rec_bc.rearrange("p c w n -> p (c w) n"),
                        op=mybir.AluOpType.mult,
                    )
                    # odd-window rows relocated to partition base 0 (the
                    # gpsimd DMA path: the HWDGE queues are congested with
                    # input/output traffic while GpSimd is idle)
                    pnT_hi = pp.tile([64, HEADS, N], BF16, tag="pnT_hi", name="pnT_hi")
                    nc.gpsimd.dma_start(out=pnT_hi, in_=pnT[64:128, :, :])
                    # X.T: [feat-in-chunk 128, chunk 4, (w, tq) 128]
                    xt = ps_x.tile([128, 4, 128], F32, tag="xt", name="xt")
                    for h in range(HEADS):
                        ch, hh = h // 2, (h % 2) * 64
                        for w in range(2):
                            psrc = pnT if w == 0 else pnT_hi
                            vsrc = yv_b if w == 0 else yv_hi_b
                            nc.tensor.matmul(
                                xt[ds(hh, 64), ch, ds(64 * w, 64)],
                                lhsT=vsrc[ds(0, 64), prb, ts(h, 64)],
                                rhs=psrc[ds(0, 64), h, :],
                                start=True,
                                stop=True,
                            )
                    xts = pp.tile([128, 4, 128], BF16, tag="xts", name="xts")
                    nc.scalar.copy(out=xts, in_=xt)
                    st2 = (xts, ab)
                else:
                    st2 = None
                st1 = nst1
                # interleave the next tile's inputs + projections between
                # this tile's pair slots
                if t + 1 < n_tt:
                    if pr == 0:
                        pend_in = emit_inputs(t + 1)
                        pend_k = emit_kproj(pend_in[1])
                    elif pr == 1:
                        pend_q = emit_qproj(pend_in[0], t + 1)
                    elif pr == 2:
                        pend_v = emit_vproj_half(pend_in[2], None, range(PAIRS))
                    elif pr == 3:
                        proj[t + 1] = (
                            pend_k,
                            pend_q,
                            pend_v,
                            emit_vhi(pend_v),
                            pend_in[3],
                        )
                        del proj[t]

    nc.compile()
    return nc


_PROGRAM_CACHE = {}


def _get_program(win_per_core):
    if win_per_core not in _PROGRAM_CACHE:
        _PROGRAM_CACHE[win_per_core] = build_program(win_per_core)
    return _PROGRAM_CACHE[win_per_core]


def _feature_major_tiles(x_flat_bf16):
    """[tok, 512] -> [n_tt, 128, 4, TT] so each T-tile block is one
    fully-contiguous transposed DMA read."""
    tok = x_flat_bf16.shape[0]
    n_tt = tok // TT
    xt = x_flat_bf16.reshape(n_tt, TT, 4, 128).transpose(0, 3, 2, 1)
    return np.ascontiguousarray(xt)


def make_in_maps(q, k, v, mask, Wq, Wk, Wv, Wp, ncores=NCORES):
    """Host-side shard + layout prep. Returns list of per-core input dicts."""
    B, n, C = q.shape
    win_pc = B // ncores
    n_tt = win_pc * n // TT
    qf = np.ascontiguousarray(q.reshape(B * n, C))
    kf = np.ascontiguousarray(k.reshape(B * n, C))
    vf = np.ascontiguousarray(v.reshape(B * n, C))

    # additive mask as a per-key-token column: [B(win), 64] ->
    # per tile [128 = (w-in-pair, tk), pair]
    madd = np.where(mask == 0, np.float32(MASK_NEG), np.float32(0.0))

    wq_t = np.ascontiguousarray((Wq.T * (SCALE * WSCALE)).astype(NP_FP8))
    wk_t = np.ascontiguousarray((Wk.T * WSCALE).astype(NP_FP8))
    wv_t = np.ascontiguousarray(Wv.T.astype(NP_BF16))
    wp_t = np.ascontiguousarray(Wp.T.astype(NP_BF16))
    ones2 = np.zeros((128, 2), NP_BF16)
    ones2[0:64, 0] = 1
    ones2[64:128, 1] = 1
    wsel = np.zeros((4, 8, 128), NP_BF16)
    for ch in range(4):
        wsel[ch, 2 * ch, 0:64] = 1
        wsel[ch, 2 * ch + 1, 64:128] = 1
    ident = np.eye(128, dtype=NP_BF16)

    tok_pc = win_pc * n
    in_maps = []
    for c in range(ncores):
        sl = slice(c * tok_pc, (c + 1) * tok_pc)
        mslice = madd[c * win_pc : (c + 1) * win_pc]  # [win_pc, 64]
        mcol = np.ascontiguousarray(
            mslice.reshape(n_tt, 4, 2, 64)
            .transpose(0, 2, 3, 1)
            .reshape(n_tt, 128, 4)
        )
        in_maps.append(
            {
                "qT": _feature_major_tiles(qf[sl].astype(NP_FP8)),
                "kT": _feature_major_tiles(kf[sl].astype(NP_FP8)),
                "vT": _feature_major_tiles(vf[sl].astype(NP_BF16)),
                "maskcol": mcol,
                "wq": wq_t,
                "wk": wk_t,
                "wv": wv_t,
                "wp": wp_t,
                "ones2": ones2,
                "wsel": wsel,
                "ident": ident,
            }
        )
    return in_maps


def _reference_numpy(q, k, v, mask, Wq, bq, Wk, bk, Wv, bv, Wp, bp):
    """Full-precision host fallback (only used for nonzero bq/bk)."""
    B, n, C = q.shape
    qh = (q.reshape(-1, C) @ Wq.T + bq).reshape(B, n, HEADS, HD).transpose(0, 2, 1, 3)
    kh = (k.reshape(-1, C) @ Wk.T + bk).reshape(B, n, HEADS, HD).transpose(0, 2, 1, 3)
    vh = (v.reshape(-1, C) @ Wv.T + bv).reshape(B, n, HEADS, HD).transpose(0, 2, 1, 3)
    s = np.einsum("bhqd,bhkd->bhqk", qh * SCALE, kh)
    s = np.where((mask[:, None, None, :] == 0), np.float32(MASK_NEG), s)
    s = s - s.max(-1, keepdims=True)
    e = np.exp(s)
    p = e / e.sum(-1, keepdims=True)
    x = np.einsum("bhqk,bhkd->bhqd", p, vh)
    x = x.transpose(0, 2, 1, 3).reshape(B, n, C)
    return (x @ Wp.T + bp).astype(np.float32)


def kernel(q, k, v, mask, Wq, bq, Wk, bk, Wv, bv, Wp, bp, trace=False):
    q = np.asarray(q, np.float32)
    k = np.asarray(k, np.float32)
    v = np.asarray(v, np.float32)
    mask = np.asarray(mask)
    Wq, Wk, Wv, Wp = (np.asarray(w, np.float32) for w in (Wq, Wk, Wv, Wp))
    bq, bk, bv, bp = (np.asarray(b, np.float32) for b in (bq, bk, bv, bp))

    if np.any(bq) or np.any(bk):
        return _reference_numpy(q, k, v, mask, Wq, bq, Wk, bk, Wv, bv, Wp, bp)

    B, n, C = q.shape
    win_pc = B // NCORES
    nc = _get_program(win_pc)
    in_maps = make_in_maps(q, k, v, mask, Wq, Wk, Wv, Wp)
    res = run_bass_kernel_spmd(
        nc, in_maps, core_ids=list(range(NCORES)), trace=trace
    )
    outs = np.concatenate(
        [np.asarray(r["out"]).astype(np.float32) for r in res.results], axis=0
    )
    outs = outs.reshape(B, n, C)
    # bv flows through attention linearly (softmax rows sum to 1); with bp it
    # folds into a single output bias.
    bout = bp + bv @ Wp.T
    if np.any(bout):
        outs = outs + bout.astype(np.float32)
    if trace:
        kernel._last_result = res
    return outs
